# revision 34
# baseline (speedup 1.0000x reference)
"""Trainium2 Bass kernel for InternalGraphConvolutionLayer.

Per node i: s_i = relu(W @ e[node_ids[i]] + sum_{edges e with segment_ids[e]==i} M @ e[neighbor_ids[e]])
result = softmax(sum_i s_i)  -> [D, 1]

Strategy (8 NeuronCores, SPMD single program; DMA-bound at ~127 us/core):
  - Nodes (segments) sharded contiguously: core c owns nodes [c*2500, (c+1)*2500).
    segment_ids is sorted, so each core's edges form one contiguous range.
  - Edge rows are gathered from HBM with one 512 B descriptor per edge (the
    cost-model/HW sweet spot: sub-512B indirect payloads are both slower per
    byte (2x small-transfer penalty) and corrupt in this NEFF path).
  - Segment-sum via one-hot matmul: edges blocked 128/partition-dim, windows of
    WSEG=64 segments; DVE builds [128, 64] one-hots (is_equal vs iota), PE
    accumulates G_block^T @ onehot into a PSUM [128d, 64seg] tile per window.
    Host pads each window to a core-uniform block count (SPMD requires an
    identical program; dummy edges get lseg=-1 -> zero one-hot row).
  - All matmul operands are float32r (same bits as f32, 2 cycles/row at
    mid-pstate vs 4 for plain f32; every producer writes f32r so the BIR
    verifier's rounding rule is satisfied - DMA-produced tensors are declared
    f32r at the dram_tensor level).
  - Gather groups are BLOCK ranges decoupled from window boundaries, small at
    the start (pipeline fill) and window-aligned-tapered at the end so the
    final chain after the last transfer is only ~4 matmuls + one small combine.
    No window may span >2 gather groups (open PSUM accumulation breaks).
  - PSUM->SBUF window copies run on the Activation engine so DVE one-hots are
    never serialized behind them.
  - Combine S = relu(W @ EnT + M @ A) is emitted incrementally as windows
    complete (per ~512-node chunk, tapering to 64 at the end); relu+row-sum
    fused on the Act engine -> per-core partial r [128, 1].
  - AllReduce r across the 8 cores + on-device softmax (fallback: host
    finalize from per-core partials).
"""

import os
import numpy as np

import concourse.bass as bass
import concourse.bacc as bacc
import concourse.tile as tile
from concourse import mybir
from concourse.bass import IndirectOffsetOnAxis, AP
from concourse.bass_utils import run_bass_kernel_spmd

D = 128
V = 100000
N = 20000
E = 640000
NCORES = 8
NSH = N // NCORES              # 2500 nodes per core
WSEG = 64                      # segments per accumulation window
NW = (NSH + WSEG - 1) // WSEG  # 40 windows per core
NBLK_NODE = (NSH + 127) // 128 # 20 node blocks
NODE_PAD = NBLK_NODE * 128     # 2560
NV = (NODE_PAD + 511) // 512   # 5 combine windows

USE_COLLECTIVE = os.environ.get("KERNEL_NO_COLLECTIVE", "") != "1"
GRPBLK = int(os.environ.get("KERNEL_GRPBLK", "34"))  # blocks per gather DMA
GBUFS = int(os.environ.get("KERNEL_GBUFS", "4"))  # gather tile double-buffering
GATHER_ONLY = os.environ.get("KERNEL_GATHER_ONLY", "") == "1"  # bench probe

LAST_EXEC_NS = None
_CACHE = {}

f32 = mybir.dt.float32
f32r = mybir.dt.float32r
i32 = mybir.dt.int32


def _build_program(blist, J, use_collective, num_devices=NCORES):
    nc = bacc.Bacc(
        "TRN2",
        target_bir_lowering=False,
        debug=False,
        num_devices=num_devices,
    )
    emb_d = nc.dram_tensor("emb", [V, D], f32, kind="ExternalInput").ap()
    ids_d = nc.dram_tensor("ids", [128, J], i32, kind="ExternalInput").ap()
    lseg_d = nc.dram_tensor("lseg", [128, J], f32, kind="ExternalInput").ap()
    nid_d = nc.dram_tensor("nid", [128, NBLK_NODE], i32, kind="ExternalInput").ap()
    wt_d = nc.dram_tensor("wt", [D, D], f32r, kind="ExternalInput").ap()
    mt_d = nc.dram_tensor("mt", [D, D], f32r, kind="ExternalInput").ap()
    idn_d = nc.dram_tensor("idn", [128, 128], f32, kind="ExternalInput").ap()
    iota_d = nc.dram_tensor("iota", [128, WSEG], f32, kind="ExternalInput").ap()
    part_d = nc.dram_tensor("part", [128, 1], f32, kind="ExternalOutput").ap()
    if use_collective:
        out_d = nc.dram_tensor("out", [1, D], f32, kind="ExternalOutput").ap()

    with tile.TileContext(nc) as tc:
        with (
            tc.tile_pool(name="const", bufs=1) as constp,
            tc.tile_pool(name="acc", bufs=1) as accp,
            tc.tile_pool(name="g", bufs=GBUFS) as gpool,
            tc.tile_pool(name="oh", bufs=8) as ohpool,
            tc.tile_pool(name="s", bufs=2) as spool,
            tc.tile_pool(name="psA", bufs=3, space="PSUM") as psA,
            tc.tile_pool(name="psT", bufs=2, space="PSUM") as psT,
            tc.tile_pool(name="psS", bufs=2, space="PSUM") as psS,
            tc.tile_pool(name="dram", bufs=1, space="DRAM") as dramp,
        ):
            ids_sb = constp.tile_from(ids_d[:])
            lseg_sb = constp.tile_from(lseg_d[:])
            iota_sb = constp.tile_from(iota_d[:])
            nid_sb = constp.tile_from(nid_d[:])
            idn_sb = constp.tile_from(idn_d[:])
            wt_sb = constp.tile_from(wt_d[:])
            mt_sb = constp.tile_from(mt_d[:])

            A_sb = accp.tile([128, NODE_PAD], f32r)
            EnT = accp.tile([128, NODE_PAD], f32r)
            r_parts = accp.tile([128, 9], f32)

            # windows only fill [0, NW*WSEG); zero the node-padding tails.
            # (memset can't write f32r directly; go through an f32 scratch +
            # tensor_copy, which is a legal f32r-rounding producer.)
            if NW * WSEG < NODE_PAD or NSH < NODE_PAD:
                zpad = accp.tile([128, NODE_PAD - min(NSH, NW * WSEG)], f32)
                nc.vector.memset(zpad[:], 0.0)
                if NW * WSEG < NODE_PAD:
                    nc.vector.tensor_copy(
                        out=A_sb[:, NW * WSEG : NODE_PAD],
                        in_=zpad[:, : NODE_PAD - NW * WSEG],
                    )
                if NSH < NODE_PAD:
                    nc.vector.tensor_copy(
                        out=EnT[:, NSH:NODE_PAD], in_=zpad[:, : NODE_PAD - NSH]
                    )

            # ---- self term: gather node embeddings, transpose to [d, n] ----
            gn = accp.tile([128, NBLK_NODE * 128], f32)
            nc.gpsimd.indirect_dma_start(
                out=gn[:],
                out_offset=None,
                in_=emb_d,
                in_offset=IndirectOffsetOnAxis(ap=nid_sb[:, :], axis=0),
            )
            for b in range(NBLK_NODE):
                pt = psT.tile([128, 128], f32)
                nc.tensor.transpose(
                    out=pt[:], in_=gn[:, b * 128 : (b + 1) * 128], identity=idn_sb[:]
                )
                ncols = min(128, NSH - b * 128)
                nc.vector.tensor_copy(
                    out=EnT[:, b * 128 : b * 128 + ncols], in_=pt[:, :ncols]
                )

            # ---- edge gather + windowed segment sum ----
            # Groups are BLOCK ranges decoupled from window boundaries: a
            # window's PSUM accumulation may span two gather groups (start/stop
            # bracket its first/last block). Small first/last groups shorten
            # the pipeline-fill and the critical tail.
            win_of_block = []   # block index -> (window, block-within-window)
            wstart = {}
            wend = {}
            for w in range(NW):
                Bw = int(blist[w])
                if Bw == 0:
                    continue
                wstart[w] = len(win_of_block)
                for b in range(Bw):
                    win_of_block.append(w)
                wend[w] = len(win_of_block)
            NB = len(win_of_block)

            # Group boundaries. Constraint: no window may span more than two
            # gather groups (3+ spans corrupt the open PSUM accumulation), so
            # all groups except the final 4-block one stay >= 20 blocks.
            first_sizes = [8, 16, 24]
            tail_sizes = [24, 20, 20, 4]
            sizes = list(first_sizes)
            rem = NB - sum(first_sizes) - sum(tail_sizes)
            assert rem > 0
            while rem > 0:
                s = min(GRPBLK, rem)
                sizes.append(s)
                rem -= s
            sizes.extend(tail_sizes)
            assert sum(sizes) == NB

            # combine-chunk boundaries taper at the end so the final chunk
            # (on the critical tail after the last gather) covers one window
            cb = [0, 512, 1024, 1536, 2048, 2304, 2432, 2496, NW * WSEG]
            cb = sorted(set(min(x, NW * WSEG) for x in cb))
            NCH = len(cb) - 1

            def emit_combine(v):
                lo = cb[v]
                hi = cb[v + 1]
                wd = hi - lo
                pS = psS.tile([128, 512], f32, tag="pS")
                nc.tensor.matmul(
                    out=pS[:, :wd], lhsT=wt_sb[:],
                    rhs=EnT[:, lo:hi],
                    start=True, stop=False,
                )
                nc.tensor.matmul(
                    out=pS[:, :wd], lhsT=mt_sb[:],
                    rhs=A_sb[:, lo:hi],
                    start=False, stop=True,
                )
                s_sb = spool.tile([128, 512], f32, tag="s")
                nc.scalar.activation(
                    out=s_sb[:, :wd],
                    in_=pS[:, :wd],
                    func=mybir.ActivationFunctionType.Relu,
                    accum_out=r_parts[:, v : v + 1],
                )

            done_w = 0
            next_chunk = 0
            ps_live = {}      # window -> psum tile with accumulation in flight
            gb0 = 0
            for gsz in sizes:
                gb1 = gb0 + gsz
                gt = gpool.tile([128, 128 * gsz], f32r, tag="gt")
                nc.gpsimd.indirect_dma_start(
                    out=gt[:],
                    out_offset=None,
                    in_=emb_d.bitcast(f32r),
                    in_offset=IndirectOffsetOnAxis(
                        ap=ids_sb[:, gb0:gb1], axis=0
                    ),
                )
                if not GATHER_ONLY:
                    b = gb0
                    while b < gb1:
                        w = win_of_block[b]
                        blo = max(wstart[w], gb0)
                        bhi = min(wend[w], gb1)
                        if w not in ps_live:
                            # one-hot for this window's blocks (all of them,
                            # built once when the window first appears)
                            Bw = wend[w] - wstart[w]
                            oh = ohpool.tile([128, WSEG * Bw], f32r, tag="oh")
                            ls = lseg_sb[:, wstart[w] : wend[w]]
                            in0 = AP(
                                ls.tensor,
                                ls.offset,
                                [list(ls.ap[0]), list(ls.ap[1]), [0, WSEG]],
                            )
                            io = iota_sb[:, :]
                            in1 = AP(
                                io.tensor,
                                io.offset,
                                [list(io.ap[0]), [0, Bw], list(io.ap[1])],
                            )
                            oh3 = oh[:].rearrange("p (b s) -> p b s", s=WSEG)
                            nc.vector.tensor_tensor(
                                out=oh3, in0=in0, in1=in1,
                                op=mybir.AluOpType.is_equal,
                            )
                            ps = psA.tile([128, WSEG], f32, tag="psw")
                            ps_live[w] = (ps, oh)
                        ps, oh = ps_live[w]
                        for bb in range(blo, bhi):
                            k = bb - wstart[w]
                            nc.tensor.matmul(
                                out=ps[:],
                                lhsT=gt[:, (bb - gb0) * 128 : (bb - gb0 + 1) * 128],
                                rhs=oh[:, k * WSEG : (k + 1) * WSEG],
                                start=(bb == wstart[w]),
                                stop=(bb == wend[w] - 1),
                            )
                        if bhi == wend[w]:
                            # copy on the (nearly idle) Activation engine: keeps
                            # DVE free for one-hots so PE never waits on the
                            # copy->onehot DVE serialization
                            nc.scalar.activation(
                                out=A_sb[:, w * WSEG : (w + 1) * WSEG],
                                in_=ps[:],
                                func=mybir.ActivationFunctionType.Copy,
                            )
                            del ps_live[w]
                            done_w += 1
                            while (
                                next_chunk < NCH
                                and done_w * WSEG >= cb[next_chunk + 1]
                            ):
                                emit_combine(next_chunk)
                                next_chunk += 1
                        b = bhi
                gb0 = gb1

            while next_chunk < NCH:
                emit_combine(next_chunk)
                next_chunk += 1
            r = accp.tile([128, 1], f32)
            nc.vector.reduce_sum(r[:], r_parts[:, :NCH], axis=mybir.AxisListType.X)
            nc.sync.dma_start(part_d[:], r[:])

            if use_collective:
                cin = dramp.tile([128, 1], f32)
                cout = dramp.tile([128, 1], f32)
                nc.gpsimd.dma_start(cin[:], r[:])
                nc.gpsimd.collective_compute(
                    "AllReduce",
                    mybir.AluOpType.add,
                    replica_groups=[list(range(NCORES))],
                    ins=[cin.opt()],
                    outs=[cout.opt()],
                )
                rg = accp.tile([128, 1], f32)
                nc.sync.dma_start(rg[:], cout[:])
                # softmax over the partition dim: transpose to a [1, 128] row
                ptr = psT.tile([128, 128], f32, tag="pt")
                nc.tensor.transpose(out=ptr[:1, :128], in_=rg[:, :1], identity=idn_sb[:])
                row = accp.tile([1, 128], f32)
                nc.vector.tensor_copy(out=row[:], in_=ptr[:1, :128])
                mx = accp.tile([1, 1], f32)
                nc.vector.reduce_max(mx[:], row[:], axis=mybir.AxisListType.X)
                nmx = accp.tile([1, 1], f32)
                nc.scalar.mul(out=nmx[:], in_=mx[:], mul=-1.0)
                erow = accp.tile([1, 128], f32)
                nc.scalar.activation(
                    out=erow[:], in_=row[:],
                    func=mybir.ActivationFunctionType.Exp,
                    bias=nmx[:],
                )
                sm = accp.tile([1, 1], f32)
                nc.vector.reduce_sum(sm[:], erow[:], axis=mybir.AxisListType.X)
                inv = accp.tile([1, 1], f32)
                nc.vector.reciprocal(inv[:], sm[:])
                yrow = accp.tile([1, 128], f32)
                nc.vector.tensor_tensor(
                    out=yrow[:], in0=erow[:], in1=inv[:].to_broadcast([1, 128]),
                    op=mybir.AluOpType.mult,
                )
                nc.sync.dma_start(out_d[:], yrow[:])

    nc.compile()
    return nc


def _prep_indices(node_ids, neighbor_ids, segment_ids):
    seg = np.asarray(segment_ids).astype(np.int64).ravel()
    nbr = np.asarray(neighbor_ids).astype(np.int64).ravel()
    nid = np.asarray(node_ids).astype(np.int64).ravel()

    los = np.empty(NCORES * NW, np.int64)
    his = np.empty(NCORES * NW, np.int64)
    k = 0
    for c in range(NCORES):
        for w in range(NW):
            los[k] = c * NSH + w * WSEG
            his[k] = min(los[k] + WSEG, (c + 1) * NSH)
            k += 1
    e_lo = np.searchsorted(seg, los, side="left")
    e_hi = np.searchsorted(seg, his, side="left")
    cnt = (e_hi - e_lo).reshape(NCORES, NW)
    blist = ((cnt.max(axis=0) + 127) // 128).astype(np.int64)  # [NW]
    J = int(blist.sum())

    ids_mat = np.zeros((NCORES, 128, J), np.int32)
    lseg_mat = np.full((NCORES, 128, J), -1.0, np.float32)
    j0 = 0
    for w in range(NW):
        Bw = int(blist[w])
        if Bw == 0:
            continue
        for c in range(NCORES):
            k = c * NW + w
            el, eh = int(e_lo[k]), int(e_hi[k])
            n = eh - el
            idsw = np.zeros(Bw * 128, np.int64)
            idsw[:n] = nbr[el:eh]
            lsw = np.full(Bw * 128, -1.0, np.float32)
            lsw[:n] = (seg[el:eh] - los[k]).astype(np.float32)
            ids_mat[c, :, j0 : j0 + Bw] = idsw.reshape(Bw, 128).T
            lseg_mat[c, :, j0 : j0 + Bw] = lsw.reshape(Bw, 128).T.astype(np.float32)
        j0 += Bw

    nid_mat = np.zeros((NCORES, 128, NBLK_NODE), np.int32)
    for c in range(NCORES):
        a = np.zeros(NODE_PAD, np.int64)
        a[:NSH] = nid[c * NSH : (c + 1) * NSH]
        nid_mat[c] = a.reshape(NBLK_NODE, 128).T
    return blist, J, ids_mat, lseg_mat, nid_mat


def kernel(node_ids, neighbor_ids, segment_ids, W, M, emb):
    global LAST_EXEC_NS
    blist, J, ids_mat, lseg_mat, nid_mat = _prep_indices(
        node_ids, neighbor_ids, segment_ids
    )
    Wt = np.ascontiguousarray(np.asarray(W, np.float32).T)
    Mt = np.ascontiguousarray(np.asarray(M, np.float32).T)
    embf = np.ascontiguousarray(np.asarray(emb, np.float32))
    idn = np.eye(128, dtype=np.float32)
    iota = np.tile(np.arange(WSEG, dtype=np.float32), (128, 1))

    key = (J, tuple(int(b) for b in blist), USE_COLLECTIVE)
    if key not in _CACHE:
        _CACHE[key] = _build_program(blist, J, USE_COLLECTIVE)
    nc = _CACHE[key]

    in_maps = []
    for c in range(NCORES):
        in_maps.append(
            {
                "emb": embf,
                "ids": np.ascontiguousarray(ids_mat[c]),
                "lseg": np.ascontiguousarray(lseg_mat[c]),
                "nid": np.ascontiguousarray(nid_mat[c]),
                "wt": Wt,
                "mt": Mt,
                "idn": idn,
                "iota": iota,
            }
        )

    res = None
    last_err = None
    for _attempt in range(3):  # rare transient NRT_EXEC_UNIT_UNRECOVERABLE
        try:
            res = run_bass_kernel_spmd(nc, in_maps, core_ids=list(range(NCORES)))
            break
        except Exception as e:  # noqa: BLE001
            last_err = e
    if res is None:
        raise last_err
    LAST_EXEC_NS = res.exec_time_ns

    if USE_COLLECTIVE:
        out = np.asarray(res.results[0]["out"], np.float32).reshape(D, 1)
        return out
    # host fallback: sum per-core partials, softmax
    r = np.zeros(D, np.float64)
    for c in range(NCORES):
        r += np.asarray(res.results[c]["part"], np.float64).ravel()
    r -= r.max()
    e = np.exp(r)
    return (e / e.sum()).astype(np.float32).reshape(D, 1)



# revision 35
# speedup vs baseline: 1.0067x; 1.0067x over previous
"""Trainium2 Bass kernel for InternalGraphConvolutionLayer.

Per node i: s_i = relu(W @ e[node_ids[i]] + sum_{edges e with segment_ids[e]==i} M @ e[neighbor_ids[e]])
result = softmax(sum_i s_i)  -> [D, 1]

Strategy (8 NeuronCores, SPMD single program; DMA-bound at ~127 us/core):
  - Nodes (segments) sharded contiguously: core c owns nodes [c*2500, (c+1)*2500).
    segment_ids is sorted, so each core's edges form one contiguous range.
  - Edge rows are gathered from HBM with one 512 B descriptor per edge (the
    cost-model/HW sweet spot: sub-512B indirect payloads are both slower per
    byte (2x small-transfer penalty) and corrupt in this NEFF path).
  - Segment-sum via one-hot matmul: edges blocked 128/partition-dim, windows of
    WSEG=64 segments; DVE builds [128, 64] one-hots (is_equal vs iota), PE
    accumulates G_block^T @ onehot into a PSUM [128d, 64seg] tile per window.
    Host pads each window to a core-uniform block count (SPMD requires an
    identical program; dummy edges get lseg=-1 -> zero one-hot row).
  - All matmul operands are float32r (same bits as f32, 2 cycles/row at
    mid-pstate vs 4 for plain f32; every producer writes f32r so the BIR
    verifier's rounding rule is satisfied - DMA-produced tensors are declared
    f32r at the dram_tensor level).
  - Gather groups are BLOCK ranges decoupled from window boundaries, small at
    the start (pipeline fill) and window-aligned-tapered at the end so the
    final chain after the last transfer is only ~4 matmuls + one small combine.
    No window may span >2 gather groups (open PSUM accumulation breaks).
  - PSUM->SBUF window copies run on the Activation engine so DVE one-hots are
    never serialized behind them.
  - Combine S = relu(W @ EnT + M @ A) is emitted incrementally as windows
    complete (per ~512-node chunk, tapering to 64 at the end); relu+row-sum
    fused on the Act engine -> per-core partial r [128, 1].
  - AllReduce r across the 8 cores + on-device softmax (fallback: host
    finalize from per-core partials).
"""

import os
import numpy as np

import concourse.bass as bass
import concourse.bacc as bacc
import concourse.tile as tile
from concourse import mybir
from concourse.bass import IndirectOffsetOnAxis, AP
from concourse.bass_utils import run_bass_kernel_spmd

D = 128
V = 100000
N = 20000
E = 640000
NCORES = 8
NSH = N // NCORES              # 2500 nodes per core
WSEG = 64                      # segments per accumulation window
NW = (NSH + WSEG - 1) // WSEG  # 40 windows per core
NBLK_NODE = (NSH + 127) // 128 # 20 node blocks
NODE_PAD = NBLK_NODE * 128     # 2560
NV = (NODE_PAD + 511) // 512   # 5 combine windows

USE_COLLECTIVE = os.environ.get("KERNEL_NO_COLLECTIVE", "") != "1"
GRPBLK = int(os.environ.get("KERNEL_GRPBLK", "34"))  # blocks per gather DMA
GBUFS = int(os.environ.get("KERNEL_GBUFS", "4"))  # gather tile double-buffering
GATHER_ONLY = os.environ.get("KERNEL_GATHER_ONLY", "") == "1"  # bench probe

LAST_EXEC_NS = None
_CACHE = {}

f32 = mybir.dt.float32
f32r = mybir.dt.float32r
i32 = mybir.dt.int32


def _build_program(blist, J, use_collective, num_devices=NCORES):
    nc = bacc.Bacc(
        "TRN2",
        target_bir_lowering=False,
        debug=False,
        num_devices=num_devices,
    )
    emb_d = nc.dram_tensor("emb", [V, D], f32, kind="ExternalInput").ap()
    ids_d = nc.dram_tensor("ids", [128, J], i32, kind="ExternalInput").ap()
    lseg_d = nc.dram_tensor("lseg", [128, J], f32, kind="ExternalInput").ap()
    nid_d = nc.dram_tensor("nid", [128, NBLK_NODE], i32, kind="ExternalInput").ap()
    wt_d = nc.dram_tensor("wt", [D, D], f32r, kind="ExternalInput").ap()
    mt_d = nc.dram_tensor("mt", [D, D], f32r, kind="ExternalInput").ap()
    idn_d = nc.dram_tensor("idn", [128, 128], f32, kind="ExternalInput").ap()
    iota_d = nc.dram_tensor("iota", [128, WSEG], f32, kind="ExternalInput").ap()
    part_d = nc.dram_tensor("part", [128, 1], f32, kind="ExternalOutput").ap()
    if use_collective:
        out_d = nc.dram_tensor("out", [1, D], f32, kind="ExternalOutput").ap()

    with tile.TileContext(nc) as tc:
        with (
            tc.tile_pool(name="const", bufs=1) as constp,
            tc.tile_pool(name="acc", bufs=1) as accp,
            tc.tile_pool(name="g", bufs=GBUFS) as gpool,
            tc.tile_pool(name="oh", bufs=8) as ohpool,
            tc.tile_pool(name="s", bufs=2) as spool,
            tc.tile_pool(name="psA", bufs=3, space="PSUM") as psA,
            tc.tile_pool(name="psT", bufs=2, space="PSUM") as psT,
            tc.tile_pool(name="psS", bufs=2, space="PSUM") as psS,
            tc.tile_pool(name="dram", bufs=1, space="DRAM") as dramp,
        ):
            ids_sb = constp.tile_from(ids_d[:])
            lseg_sb = constp.tile_from(lseg_d[:])
            iota_sb = constp.tile_from(iota_d[:])
            nid_sb = constp.tile_from(nid_d[:])
            idn_sb = constp.tile_from(idn_d[:])
            wt_sb = constp.tile_from(wt_d[:])
            mt_sb = constp.tile_from(mt_d[:])

            A_sb = accp.tile([128, NODE_PAD], f32r)
            EnT = accp.tile([128, NODE_PAD], f32r)
            r_parts = accp.tile([128, 9], f32)

            # windows only fill [0, NW*WSEG); zero the node-padding tails.
            # (memset can't write f32r directly; go through an f32 scratch +
            # tensor_copy, which is a legal f32r-rounding producer.)
            if NW * WSEG < NODE_PAD or NSH < NODE_PAD:
                zpad = accp.tile([128, NODE_PAD - min(NSH, NW * WSEG)], f32)
                nc.vector.memset(zpad[:], 0.0)
                if NW * WSEG < NODE_PAD:
                    nc.vector.tensor_copy(
                        out=A_sb[:, NW * WSEG : NODE_PAD],
                        in_=zpad[:, : NODE_PAD - NW * WSEG],
                    )
                if NSH < NODE_PAD:
                    nc.vector.tensor_copy(
                        out=EnT[:, NSH:NODE_PAD], in_=zpad[:, : NODE_PAD - NSH]
                    )

            # ---- self term: gather node embeddings, transpose to [d, n] ----
            gn = accp.tile([128, NBLK_NODE * 128], f32)
            nc.gpsimd.indirect_dma_start(
                out=gn[:],
                out_offset=None,
                in_=emb_d,
                in_offset=IndirectOffsetOnAxis(ap=nid_sb[:, :], axis=0),
            )
            for b in range(NBLK_NODE):
                pt = psT.tile([128, 128], f32)
                nc.tensor.transpose(
                    out=pt[:], in_=gn[:, b * 128 : (b + 1) * 128], identity=idn_sb[:]
                )
                ncols = min(128, NSH - b * 128)
                nc.vector.tensor_copy(
                    out=EnT[:, b * 128 : b * 128 + ncols], in_=pt[:, :ncols]
                )

            # ---- edge gather + windowed segment sum ----
            # Groups are BLOCK ranges decoupled from window boundaries: a
            # window's PSUM accumulation may span two gather groups (start/stop
            # bracket its first/last block). Small first/last groups shorten
            # the pipeline-fill and the critical tail.
            win_of_block = []   # block index -> (window, block-within-window)
            wstart = {}
            wend = {}
            for w in range(NW):
                Bw = int(blist[w])
                if Bw == 0:
                    continue
                wstart[w] = len(win_of_block)
                for b in range(Bw):
                    win_of_block.append(w)
                wend[w] = len(win_of_block)
            NB = len(win_of_block)

            # Group boundaries. Constraint: no window may span more than two
            # gather groups (3+ spans corrupt the open PSUM accumulation).
            # Fillers stay >= 17 (> max blocks/window - 1); the last 3 windows
            # are halved at window-aligned cuts, so every tail group lies
            # inside one window and the final transfer is tiny.
            tail_ws = sorted(wstart)[-3:]
            cutpts = []
            for w in tail_ws:
                s0, e0 = wstart[w], wend[w]
                cutpts.append(s0)
                if e0 - s0 >= 2:
                    cutpts.append(s0 + (e0 - s0) // 2)
            cutpts.append(NB)
            sizes = [8, 16, 24]
            rem = cutpts[0] - sum(sizes)
            assert rem >= GRPBLK
            while rem > 0:
                s = min(GRPBLK, rem)
                if 0 < rem - s < 17:
                    s = rem - 17
                sizes.append(s)
                rem -= s
            for a, b in zip(cutpts, cutpts[1:]):
                if b > a:
                    sizes.append(b - a)
            assert sum(sizes) == NB and all(s > 0 for s in sizes)

            # combine-chunk boundaries taper at the end so the final chunk
            # (on the critical tail after the last gather) covers one window
            cb = [0, 512, 1024, 1536, 2048, 2304, 2432, 2496, NW * WSEG]
            cb = sorted(set(min(x, NW * WSEG) for x in cb))
            NCH = len(cb) - 1

            def emit_combine(v):
                lo = cb[v]
                hi = cb[v + 1]
                wd = hi - lo
                pS = psS.tile([128, 512], f32, tag="pS")
                nc.tensor.matmul(
                    out=pS[:, :wd], lhsT=wt_sb[:],
                    rhs=EnT[:, lo:hi],
                    start=True, stop=False,
                )
                nc.tensor.matmul(
                    out=pS[:, :wd], lhsT=mt_sb[:],
                    rhs=A_sb[:, lo:hi],
                    start=False, stop=True,
                )
                s_sb = spool.tile([128, 512], f32, tag="s")
                nc.scalar.activation(
                    out=s_sb[:, :wd],
                    in_=pS[:, :wd],
                    func=mybir.ActivationFunctionType.Relu,
                    accum_out=r_parts[:, v : v + 1],
                )

            done_w = 0
            next_chunk = 0
            ps_live = {}      # window -> psum tile with accumulation in flight
            gb0 = 0
            for gsz in sizes:
                gb1 = gb0 + gsz
                gt = gpool.tile([128, 128 * gsz], f32r, tag="gt")
                nc.gpsimd.indirect_dma_start(
                    out=gt[:],
                    out_offset=None,
                    in_=emb_d.bitcast(f32r),
                    in_offset=IndirectOffsetOnAxis(
                        ap=ids_sb[:, gb0:gb1], axis=0
                    ),
                )
                if not GATHER_ONLY:
                    b = gb0
                    while b < gb1:
                        w = win_of_block[b]
                        blo = max(wstart[w], gb0)
                        bhi = min(wend[w], gb1)
                        if w not in ps_live:
                            # one-hot for this window's blocks (all of them,
                            # built once when the window first appears)
                            Bw = wend[w] - wstart[w]
                            oh = ohpool.tile([128, WSEG * Bw], f32r, tag="oh")
                            ls = lseg_sb[:, wstart[w] : wend[w]]
                            in0 = AP(
                                ls.tensor,
                                ls.offset,
                                [list(ls.ap[0]), list(ls.ap[1]), [0, WSEG]],
                            )
                            io = iota_sb[:, :]
                            in1 = AP(
                                io.tensor,
                                io.offset,
                                [list(io.ap[0]), [0, Bw], list(io.ap[1])],
                            )
                            oh3 = oh[:].rearrange("p (b s) -> p b s", s=WSEG)
                            nc.vector.tensor_tensor(
                                out=oh3, in0=in0, in1=in1,
                                op=mybir.AluOpType.is_equal,
                            )
                            ps = psA.tile([128, WSEG], f32, tag="psw")
                            ps_live[w] = (ps, oh)
                        ps, oh = ps_live[w]
                        for bb in range(blo, bhi):
                            k = bb - wstart[w]
                            nc.tensor.matmul(
                                out=ps[:],
                                lhsT=gt[:, (bb - gb0) * 128 : (bb - gb0 + 1) * 128],
                                rhs=oh[:, k * WSEG : (k + 1) * WSEG],
                                start=(bb == wstart[w]),
                                stop=(bb == wend[w] - 1),
                            )
                        if bhi == wend[w]:
                            # copy on the (nearly idle) Activation engine: keeps
                            # DVE free for one-hots so PE never waits on the
                            # copy->onehot DVE serialization
                            nc.scalar.activation(
                                out=A_sb[:, w * WSEG : (w + 1) * WSEG],
                                in_=ps[:],
                                func=mybir.ActivationFunctionType.Copy,
                            )
                            del ps_live[w]
                            done_w += 1
                            while (
                                next_chunk < NCH
                                and done_w * WSEG >= cb[next_chunk + 1]
                            ):
                                emit_combine(next_chunk)
                                next_chunk += 1
                        b = bhi
                gb0 = gb1

            while next_chunk < NCH:
                emit_combine(next_chunk)
                next_chunk += 1
            r = accp.tile([128, 1], f32)
            nc.vector.reduce_sum(r[:], r_parts[:, :NCH], axis=mybir.AxisListType.X)
            nc.sync.dma_start(part_d[:], r[:])

            if use_collective:
                cin = dramp.tile([128, 1], f32)
                cout = dramp.tile([128, 1], f32)
                nc.gpsimd.dma_start(cin[:], r[:])
                nc.gpsimd.collective_compute(
                    "AllReduce",
                    mybir.AluOpType.add,
                    replica_groups=[list(range(NCORES))],
                    ins=[cin.opt()],
                    outs=[cout.opt()],
                )
                rg = accp.tile([128, 1], f32)
                nc.sync.dma_start(rg[:], cout[:])
                # softmax over the partition dim: transpose to a [1, 128] row
                ptr = psT.tile([128, 128], f32, tag="pt")
                nc.tensor.transpose(out=ptr[:1, :128], in_=rg[:, :1], identity=idn_sb[:])
                row = accp.tile([1, 128], f32)
                nc.vector.tensor_copy(out=row[:], in_=ptr[:1, :128])
                mx = accp.tile([1, 1], f32)
                nc.vector.reduce_max(mx[:], row[:], axis=mybir.AxisListType.X)
                nmx = accp.tile([1, 1], f32)
                nc.scalar.mul(out=nmx[:], in_=mx[:], mul=-1.0)
                erow = accp.tile([1, 128], f32)
                nc.scalar.activation(
                    out=erow[:], in_=row[:],
                    func=mybir.ActivationFunctionType.Exp,
                    bias=nmx[:],
                )
                sm = accp.tile([1, 1], f32)
                nc.vector.reduce_sum(sm[:], erow[:], axis=mybir.AxisListType.X)
                inv = accp.tile([1, 1], f32)
                nc.vector.reciprocal(inv[:], sm[:])
                yrow = accp.tile([1, 128], f32)
                nc.vector.tensor_tensor(
                    out=yrow[:], in0=erow[:], in1=inv[:].to_broadcast([1, 128]),
                    op=mybir.AluOpType.mult,
                )
                nc.sync.dma_start(out_d[:], yrow[:])

    nc.compile()
    return nc


def _prep_indices(node_ids, neighbor_ids, segment_ids):
    seg = np.asarray(segment_ids).astype(np.int64).ravel()
    nbr = np.asarray(neighbor_ids).astype(np.int64).ravel()
    nid = np.asarray(node_ids).astype(np.int64).ravel()

    los = np.empty(NCORES * NW, np.int64)
    his = np.empty(NCORES * NW, np.int64)
    k = 0
    for c in range(NCORES):
        for w in range(NW):
            los[k] = c * NSH + w * WSEG
            his[k] = min(los[k] + WSEG, (c + 1) * NSH)
            k += 1
    e_lo = np.searchsorted(seg, los, side="left")
    e_hi = np.searchsorted(seg, his, side="left")
    cnt = (e_hi - e_lo).reshape(NCORES, NW)
    blist = ((cnt.max(axis=0) + 127) // 128).astype(np.int64)  # [NW]
    J = int(blist.sum())

    ids_mat = np.zeros((NCORES, 128, J), np.int32)
    lseg_mat = np.full((NCORES, 128, J), -1.0, np.float32)
    j0 = 0
    for w in range(NW):
        Bw = int(blist[w])
        if Bw == 0:
            continue
        for c in range(NCORES):
            k = c * NW + w
            el, eh = int(e_lo[k]), int(e_hi[k])
            n = eh - el
            idsw = np.zeros(Bw * 128, np.int64)
            idsw[:n] = nbr[el:eh]
            lsw = np.full(Bw * 128, -1.0, np.float32)
            lsw[:n] = (seg[el:eh] - los[k]).astype(np.float32)
            ids_mat[c, :, j0 : j0 + Bw] = idsw.reshape(Bw, 128).T
            lseg_mat[c, :, j0 : j0 + Bw] = lsw.reshape(Bw, 128).T.astype(np.float32)
        j0 += Bw

    nid_mat = np.zeros((NCORES, 128, NBLK_NODE), np.int32)
    for c in range(NCORES):
        a = np.zeros(NODE_PAD, np.int64)
        a[:NSH] = nid[c * NSH : (c + 1) * NSH]
        nid_mat[c] = a.reshape(NBLK_NODE, 128).T
    return blist, J, ids_mat, lseg_mat, nid_mat


def kernel(node_ids, neighbor_ids, segment_ids, W, M, emb):
    global LAST_EXEC_NS
    blist, J, ids_mat, lseg_mat, nid_mat = _prep_indices(
        node_ids, neighbor_ids, segment_ids
    )
    Wt = np.ascontiguousarray(np.asarray(W, np.float32).T)
    Mt = np.ascontiguousarray(np.asarray(M, np.float32).T)
    embf = np.ascontiguousarray(np.asarray(emb, np.float32))
    idn = np.eye(128, dtype=np.float32)
    iota = np.tile(np.arange(WSEG, dtype=np.float32), (128, 1))

    key = (J, tuple(int(b) for b in blist), USE_COLLECTIVE)
    if key not in _CACHE:
        _CACHE[key] = _build_program(blist, J, USE_COLLECTIVE)
    nc = _CACHE[key]

    in_maps = []
    for c in range(NCORES):
        in_maps.append(
            {
                "emb": embf,
                "ids": np.ascontiguousarray(ids_mat[c]),
                "lseg": np.ascontiguousarray(lseg_mat[c]),
                "nid": np.ascontiguousarray(nid_mat[c]),
                "wt": Wt,
                "mt": Mt,
                "idn": idn,
                "iota": iota,
            }
        )

    res = None
    last_err = None
    for _attempt in range(3):  # rare transient NRT_EXEC_UNIT_UNRECOVERABLE
        try:
            res = run_bass_kernel_spmd(nc, in_maps, core_ids=list(range(NCORES)))
            break
        except Exception as e:  # noqa: BLE001
            last_err = e
    if res is None:
        raise last_err
    LAST_EXEC_NS = res.exec_time_ns

    if USE_COLLECTIVE:
        out = np.asarray(res.results[0]["out"], np.float32).reshape(D, 1)
        return out
    # host fallback: sum per-core partials, softmax
    r = np.zeros(D, np.float64)
    for c in range(NCORES):
        r += np.asarray(res.results[c]["part"], np.float64).ravel()
    r -= r.max()
    e = np.exp(r)
    return (e / e.sum()).astype(np.float32).reshape(D, 1)

